# revision 2
# baseline (speedup 1.0000x reference)
"""Trainium2 Bass kernel: 2D valid cross-correlation (4096x4096 image,
15x15 kernel), output columns sharded across 8 NeuronCores (512 each).

Per core the PE runs 4-way tiled (2x2 grid of 64x64 subarrays via
tile_position): each subarray computes an independent row-tile's Toeplitz
conv with K=63 contraction — band density 15/64 vs 15/128 for a full
128x128 formulation, with all four 512-column bf16 streams running
concurrently at the array rate (~216ns/round; the per-matmul LDWEIGHTS,
~58ns on its own port, just fits underneath).

Per pass (4 row-tiles x 49 output rows = 196 rows):
  tile (g, c) in {0,1}^2 at tile_position (64g, 64c), row-tile k = 4s+2c+g
  moving  = xs[64g:64g+63, 526c + b : 526c + b + 512]   (bf16)
  lhsT    = wt[64g:64g+63, 49b : 49b + 49]              (bf16 Toeplitz band)
  psum    = psums[g][64c:64c+49, :512]  (bank g, partition half c)
  15 accumulating matmuls (b = 0..14); K stays 63 even for the short last
  tile (zero band rows nullify out-of-range inputs) so the PE never
  switches tiling mode.

The host pre-tiles X into the exact [128, 2*526] bf16 SBUF layouts (so
every DMA is chunky and contiguous) and de-tiles the [128, 2*512] bf16
output blocks; bf16 keeps rel err ~3e-3, well under the 2e-2 gate.
84 row-tiles = 21 full passes; measured ~97.0us HW exec (was 154.6us
for the fp32r 128x128 Toeplitz baseline).
"""

import numpy as np
import ml_dtypes

import concourse.bass as bass
import concourse.mybir as mybir
import concourse.tile as tile
from concourse import bacc
from concourse.bass_utils import run_bass_kernel_spmd

BF16 = ml_dtypes.bfloat16

H, W = 4096, 4096
KH, KW = 15, 15
OH, OW = H - KH + 1, W - KW + 1  # 4082 x 4082

NCORES = 8
NT = 512                      # output cols per core
IN_COLS = NT + KW - 1         # 526

MT = 49                       # output rows per row-tile
KT = 63                       # input rows per row-tile (MT + 14)
TPP = 4                       # row-tiles per pass (2 row-groups x 2 col-groups)
NROWTILES = (OH + MT - 1) // MT          # 84 = 21*4 (last tile: 15 rows)
NPASS = (NROWTILES + TPP - 1) // TPP     # 21, all full

F32 = mybir.dt.float32
DT = mybir.dt.bfloat16


def _row_tile_dims(k: int) -> tuple[int, int]:
    """(M, K) for row-tile k. K stays KT even for the short last tile —
    the zero rows of the Toeplitz band nullify the out-of-range inputs,
    and a uniform K keeps the PE in one (64,64) tiling mode."""
    m = min(MT, OH - k * MT)
    return m, KT


def _build_program():
    nc = bacc.Bacc("TRN2", target_bir_lowering=False, debug=False)
    x = nc.dram_tensor("x", [NPASS * 128, 2 * IN_COLS], DT, kind="ExternalInput").ap()
    wt = nc.dram_tensor("wt", [128, KW * MT], DT, kind="ExternalInput").ap()
    out = nc.dram_tensor("out", [NPASS * 128, 2 * NT], DT, kind="ExternalOutput").ap()

    with tile.TileContext(nc) as tc:
        with (
            tc.tile_pool(name="wpool", bufs=1) as wpool,
            tc.tile_pool(name="xpool", bufs=4) as xpool,
            tc.tile_pool(name="opool", bufs=3) as opool,
            tc.tile_pool(name="dpool", bufs=1) as dpool,
            tc.tile_pool(name="ppool", bufs=3, space="PSUM") as ppool,
        ):
            # HAM pre-warm on tile (0,0) in 64x64 mode while first DMAs land.
            dz = dpool.tile([64, 128], F32, tag="dz")
            nc.vector.memset(dz[:], 0)
            dummy = dpool.tile([64, 128], DT, tag="dummy")
            nc.vector.tensor_copy(dummy[:], dz[:])
            dacc = ppool.tile([128, NT], F32, tag="pb0", name="dacc")
            for _ in range(40):
                nc.tensor.matmul(
                    dacc[0:64, 0:128],
                    dummy[:, 0:64],
                    dummy[:, 0:128],
                    start=True,
                    stop=True,
                    tile_position=(0, 0),
                )

            wtile = wpool.tile([128, KW * MT], DT, tag="wt")
            nc.scalar.dma_start(wtile[:], wt[:])

            dma_engines = (nc.sync, nc.scalar, nc.gpsimd, nc.sync)

            for s in range(NPASS):
                xs = xpool.tile([128, 2 * IN_COLS], DT, tag="xs")
                for q in range(4):
                    dma_engines[q].dma_start(
                        xs[32 * q : 32 * q + 32, :],
                        x[s * 128 + 32 * q : s * 128 + 32 * q + 32, :],
                    )

                psums = [
                    ppool.tile([128, NT], F32, tag=f"pb{g}", name=f"psum{g}")
                    for g in range(2)
                ]
                for b in range(KW):
                    for c in range(2):
                        for g in range(2):
                            k = TPP * s + 2 * c + g
                            if k >= NROWTILES:
                                continue
                            M, K = _row_tile_dims(k)
                            nc.tensor.matmul(
                                psums[g][64 * c : 64 * c + M, :],
                                wtile[64 * g : 64 * g + K, MT * b : MT * b + M],
                                xs[64 * g : 64 * g + K, IN_COLS * c + b : IN_COLS * c + b + NT],
                                start=(b == 0),
                                stop=(b == KW - 1),
                                tile_position=(64 * g, 64 * c),
                            )

                ot = opool.tile([128, 2 * NT], DT, tag="ot")
                n_banks = min(2, NROWTILES - TPP * s)
                for g in range(n_banks):
                    dst = ot[:, NT * g : NT * g + NT]
                    if g == 1:
                        nc.scalar.copy(dst, psums[g][:])
                    else:
                        nc.vector.tensor_copy(dst, psums[g][:])
                ocols = n_banks * NT
                nc.sync.dma_start(
                    out[s * 128 : s * 128 + 64, :ocols], ot[0:64, :ocols]
                )
                nc.gpsimd.dma_start(
                    out[s * 128 + 64 : s * 128 + 128, :ocols], ot[64:128, :ocols]
                )
    nc.finalize()
    return nc


def _toeplitz_pack(weight: np.ndarray) -> np.ndarray:
    """[128, 15*50] bf16: partition 64g+r, col 50b+m = w[r-m, b] (replicated
    over the 2 row-groups)."""
    wt64 = np.zeros((64, KW * MT), dtype=np.float32)
    r = np.arange(64)[:, None]
    m = np.arange(MT)[None, :]
    a = r - m
    valid = (a >= 0) & (a < KH)
    av = np.where(valid, a, 0)
    for b in range(KW):
        wt64[:, b * MT : (b + 1) * MT] = np.where(valid, weight[av, b], 0.0)
    return np.tile(wt64, (2, 1)).astype(BF16)


def _pretile_x(Xc: np.ndarray) -> np.ndarray:
    """Xc: [H, IN_COLS] fp32 (core's column slice, zero-padded).
    Returns [NPASS*128, 2*IN_COLS] bf16: partition 64g+pr, block c =
    X[50*(4s+2c+g) + pr, :]."""
    Xpad = np.zeros((NPASS * TPP * MT + 64, IN_COLS), dtype=np.float32)
    Xpad[:H] = Xc
    s_ = np.arange(NPASS)[:, None, None]             # [S,1,1]
    c_ = np.arange(2)[None, :, None]                 # [1,2,1]
    p_ = np.arange(128)[None, None, :]               # [1,1,128]
    g_ = p_ // 64
    pr = p_ % 64
    row = MT * (TPP * s_ + 2 * c_ + g_) + pr         # [S,2,128]
    row = np.minimum(row, Xpad.shape[0] - 1)
    blk = Xpad[row]                                  # [S,2,128,IN_COLS]
    blk = np.transpose(blk, (0, 2, 1, 3))            # [S,128,2,IN_COLS]
    return np.ascontiguousarray(
        blk.reshape(NPASS * 128, 2 * IN_COLS)
    ).astype(BF16)


def kernel(X: np.ndarray, weight: np.ndarray, bias: np.ndarray) -> np.ndarray:
    X = np.ascontiguousarray(X, dtype=np.float32)
    weight = np.ascontiguousarray(weight, dtype=np.float32)
    bias = np.asarray(bias, dtype=np.float32)

    wt = _toeplitz_pack(weight)

    in_maps = []
    for n in range(NCORES):
        c0 = n * NT
        c1 = min(c0 + IN_COLS, W)
        Xc = np.zeros((H, IN_COLS), dtype=np.float32)
        Xc[:, : c1 - c0] = X[:, c0:c1]
        in_maps.append({"x": _pretile_x(Xc), "wt": wt})

    nc = _build_program()
    res = run_bass_kernel_spmd(nc, in_maps, core_ids=list(range(NCORES)))
    global _last_results
    _last_results = res

    out = np.empty((OH, OW), dtype=np.float32)
    for n in range(NCORES):
        c0 = n * NT
        ncols = min(NT, OW - c0)
        buf = (
            res.results[n]["out"]
            .reshape(NPASS, 128, 2, NT)
            .transpose(0, 2, 1, 3)
            .astype(np.float32)
        )
        for s in range(NPASS):
            for g in range(2):
                for c in range(2):
                    k = TPP * s + 2 * c + g
                    if k >= NROWTILES:
                        continue
                    M, _ = _row_tile_dims(k)
                    out[MT * k : MT * k + M, c0 : c0 + ncols] = buf[
                        s, g, 64 * c : 64 * c + M, :ncols
                    ]

    b0 = float(bias.reshape(-1)[0]) if bias.size else 0.0
    if b0 != 0.0:
        out += b0
    return out
